# revision 2
# baseline (speedup 1.0000x reference)
"""LRU (linear recurrent unit) Trainium2 kernel.

h_t = lam * h_{t-1} + gam * x_t  per channel, lam = exp(-exp(nu_logs)),
gam = sqrt(1 - lam^2).  Uses h = gam * s with s_t = lam*s_{t-1} + x_t so the
gamma scale happens once on the scalar engine after the scan.

Sharding (per the b*d-parallel recurrence structure): 8 cores = 8 channel
groups of 128 channels, each core scans all 4 batches over the full 8192
sequence.  Host-side sharding lays each core's slice out channel-major
([128, B, I]) so every DMA is the canonical contiguous-per-partition
pattern; the gather transposes back.  No cross-core communication.

The kernel is HBM-bound (per-core floor = bytes/358 GB/s), so HBM I/O is
fp16: the host downcasts x, the DVE scan reads fp16 and keeps its carry in
fp32 (the TensorTensorScan state is fp32 regardless of operand dtype, and
tiles chain through the f32 scan output), and the scalar engine fuses the
gamma scale with the fp16 downcast of the output.  That halves HBM traffic
vs f32 I/O; quantization error ~1e-3 relative, well inside the 2e-2 gate.

On-chip per core: partition p = channel, free dim = time.  One
tensor_tensor_scan (state = lam*state + x, the native DVE recurrence) per
[128, TT] tile, chained across tiles via `initial`; gamma scale on the
scalar engine; loads issued on the SP HWDGE ring, stores on the ACT ring so
stores never block load prefetch.
"""

import numpy as np
from contextlib import ExitStack

import concourse.bass as bass
import concourse.tile as tile
from concourse import bacc, mybir
from concourse.bass_utils import run_bass_kernel_spmd

B, I, D = 4, 8192, 1024
P = 128             # channels per core = SBUF partitions
TT = 4096           # seq steps per tile
NCHUNK = I // TT    # seq chunks per batch

F32 = mybir.dt.float32
F16 = mybir.dt.float16


def _lru_kernel(ctx: ExitStack, tc: tile.TileContext, xs, nu, ys):
    nc = tc.nc
    const = ctx.enter_context(tc.tile_pool(name="const", bufs=1))
    xpool = ctx.enter_context(tc.tile_pool(name="x", bufs=3))
    spool = ctx.enter_context(tc.tile_pool(name="s", bufs=3))
    hpool = ctx.enter_context(tc.tile_pool(name="h", bufs=3))

    # --- per-channel decay lam and input scale gam, [P, 1] ---
    nu1 = const.tile([P, 1], F32)
    nc.sync.dma_start(out=nu1[:], in_=nu.rearrange("(p o) -> p o", o=1))
    nus = const.tile([P, 1], F32)
    nc.scalar.activation(nus[:], nu1[:], mybir.ActivationFunctionType.Exp)
    lam = const.tile([P, 1], F32)
    nc.scalar.activation(lam[:], nus[:], mybir.ActivationFunctionType.Exp,
                         scale=-1.0)
    lam2 = const.tile([P, 1], F32)
    nc.vector.tensor_mul(lam2[:], lam[:], lam[:])
    gam = const.tile([P, 1], F32)
    nc.scalar.activation(gam[:], lam2[:], mybir.ActivationFunctionType.Sqrt,
                         scale=-1.0, bias=1.0)

    for b in range(B):
        s_prev = None
        for i in range(NCHUNK):
            x_t = xpool.tile([P, TT], F16)
            nc.sync.dma_start(out=x_t[:], in_=xs[:, b, i * TT:(i + 1) * TT])
            s_t = spool.tile([P, TT], F32)
            init = 0.0 if i == 0 else s_prev[:, TT - 1:TT]
            nc.vector.tensor_tensor_scan(
                out=s_t[:],
                data0=lam[:, 0:1].broadcast_to([P, TT]),
                data1=x_t[:],
                initial=init,
                op0=mybir.AluOpType.mult,
                op1=mybir.AluOpType.add,
            )
            h_t = hpool.tile([P, TT], F16)
            nc.scalar.activation(h_t[:], s_t[:],
                                 mybir.ActivationFunctionType.Copy,
                                 scale=gam[:, 0:1])
            # store on the ACT HWDGE ring; loads stay on the SP ring
            nc.scalar.dma_start(out=ys[:, b, i * TT:(i + 1) * TT], in_=h_t[:])
            s_prev = s_t


_NC = None


def _build():
    global _NC
    if _NC is not None:
        return _NC
    nc = bacc.Bacc("TRN2", target_bir_lowering=False, debug=False,
                   num_devices=8)
    xs = nc.dram_tensor("xs", [P, B, I], F16, kind="ExternalInput").ap()
    nu = nc.dram_tensor("nu", [P], F32, kind="ExternalInput").ap()
    ys = nc.dram_tensor("ys", [P, B, I], F16, kind="ExternalOutput").ap()
    with tile.TileContext(nc) as tc:
        with ExitStack() as ctx:
            _lru_kernel(ctx, tc, xs, nu, ys)
    nc.compile()
    _NC = nc
    return nc


def _in_maps(x, nu_logs):
    # x: [B, I, D] -> per core c: [P, B, I] slice of channels (host-side
    # shard + layout change so device DMAs are contiguous per partition;
    # fp16 downcast here halves device HBM traffic)
    xt = np.transpose(x, (2, 0, 1)).astype(np.float16)  # [D, B, I]
    maps = []
    for c in range(8):
        maps.append({
            "xs": xt[c * P:(c + 1) * P],
            "nu": np.ascontiguousarray(nu_logs[c * P:(c + 1) * P],
                                       dtype=np.float32),
        })
    return maps


def kernel(x, nu_logs, _trace=False, **_tk):
    x = np.asarray(x, dtype=np.float32)
    nu_logs = np.asarray(nu_logs, dtype=np.float32)
    nc = _build()
    r = run_bass_kernel_spmd(nc, _in_maps(x, nu_logs), list(range(8)),
                             trace=_trace, **_tk)
    out = np.empty((D, B, I), np.float16)
    for c in range(8):
        out[c * P:(c + 1) * P] = r.results[c]["ys"]
    out = np.transpose(out, (1, 2, 0)).astype(np.float32)  # [B, I, D]
    if _trace:
        return out, r
    return out
